# revision 7
# baseline (speedup 1.0000x reference)
"""Trainium2 Bass kernel for nn_DynamicDictionaryLearning (vq_codebook).

Computation (full shapes):
    query_embed = (basic_queries @ W_mlp + b_mlp).reshape(T, R, D)    # (T, R*D)
    dynamic_queries = einsum('btr,trd->btd', query_weights, query_embed)
    basic_expanded  = broadcast(basic_queries, (B, T, D))

Sharding (8 NeuronCores, one chip):
    Stage 1 (token-MLP expansion) is tensor-sharded over the R*D output dim:
    core r computes qe_r = basic_queries @ W_mlp[:, r*D:(r+1)*D] + b_r, i.e.
    the r-th slice of the expanded dictionary for ALL tokens.  This reads
    only 1/8th of W_mlp per core (the dominant input traffic).

    An on-chip AllToAll then redistributes qe so that core c holds all R
    slices for its 128-token slice.  Stage 2 (the weighted sum over R) runs
    as dense PE matmuls with block-diagonal qw tiles: contraction dim packs
    (16 tokens x 8 r) = 128, output dim packs (16 tokens x 8 batch) = 128.

    Both stages run as float32r matmuls (full fp32 data, ~1 cycle/row on
    the PE for N=512) accumulated in fp32 PSUM.  The pipeline is chunked
    over 4 D-quarters so the collectives and output DMA overlap the next
    quarter's stage-1 matmuls.

    basic_expanded is a pure broadcast of an input -> host-side view.
"""

import numpy as np

import concourse.bass as bass
import concourse.mybir as mybir
import concourse.tile as tile
from concourse import bacc
from concourse.bass_utils import run_bass_kernel_spmd

# Problem shapes (hardcoded per spec)
D = 2048
T = 1024
R = 8
B = 32
NC = 8
TS = T // NC          # 128 tokens per core (stage-2 ownership)
P = 128
KT = D // P           # 16 contraction tiles
MT = T // P           # 8 token tiles (stage 1)
NQ = 4                # D-quarters (pipeline chunks)
NW = D // NQ          # 512 = matmul free dim / PSUM bank
TG = TS // 16         # 8 token groups of 16 (stage 2)
BG = B // 8           # 4 batch groups of 8 (stage 2)

F32 = mybir.dt.float32
F32R = mybir.dt.float32r

_cache = {}


def _build_nc():
    nc = bacc.Bacc("TRN2", target_bir_lowering=False, num_devices=NC)

    bqT = nc.dram_tensor("bqT", [D, T], F32R, kind="ExternalInput")
    Wc = nc.dram_tensor("Wc", [D, D], F32R, kind="ExternalInput")
    biasr = nc.dram_tensor("biasr", [P, D], F32, kind="ExternalInput")
    Lt = nc.dram_tensor("Lt", [TG, BG, P, P], F32R, kind="ExternalInput")
    dq = nc.dram_tensor("dq", [B, TS, D], F32, kind="ExternalOutput")

    bqT_t = bqT.rearrange("(kt p) m -> kt p m", p=P)   # (16, 128, 1024)
    Wc_t = Wc.rearrange("(kt p) d -> kt p d", p=P)     # (16, 128, 2048)

    with tile.TileContext(nc) as tc:
        with (
            tc.tile_pool(name="bqp", bufs=1) as bqpool,
            tc.tile_pool(name="wp", bufs=2 * KT + 2) as wpool,
            tc.tile_pool(name="constp", bufs=1) as cpool,
            tc.tile_pool(name="lp", bufs=1) as lpool,
            tc.tile_pool(name="qep", bufs=4) as qepool,
            tc.tile_pool(name="q2p", bufs=3) as q2pool,
            tc.tile_pool(name="o2p", bufs=6) as o2pool,
            tc.tile_pool(name="ps1", bufs=3, space="PSUM") as ps1pool,
            tc.tile_pool(name="ps2", bufs=4, space="PSUM") as ps2pool,
            tc.tile_pool(name="dramp", bufs=1, space="DRAM") as dram,
        ):
            # --- preload: bq (lhsT), bias, qw block-diag tiles ---
            bq_tiles = {}
            for m in range(MT):
                for k in range(KT):
                    t = bqpool.tile([P, P], F32R, name=f"bq{k}_{m}")
                    nc.sync.dma_start(out=t, in_=bqT_t[k][:, m * P:(m + 1) * P])
                    bq_tiles[(k, m)] = t
            bias_t = cpool.tile([P, D], F32, name="bias")
            nc.sync.dma_start(out=bias_t, in_=biasr[:, :])
            l_tiles = {}
            for g in range(TG):
                for h in range(BG):
                    t = lpool.tile([P, P], F32R, name=f"L{g}_{h}")
                    nc.sync.dma_start(out=t, in_=Lt[g, h])
                    l_tiles[(g, h)] = t

            ain = [dram.tile([T, NW], F32R, name=f"ain{j}") for j in range(NQ)]
            aout = [dram.tile([T, NW], F32R, name=f"aout{j}") for j in range(NQ)]

            def stage1(j):
                w_tiles = []
                for k in range(KT):
                    t = wpool.tile([P, NW], F32R, name="w")
                    nc.sync.dma_start(out=t, in_=Wc_t[k][:, j * NW:(j + 1) * NW])
                    w_tiles.append(t)
                with nc.named_scope(f"s1_q{j}"):
                    for m in range(MT):
                        ps = ps1pool.tile([P, NW], F32, name="ps1")
                        for k in range(KT):
                            nc.tensor.matmul(
                                ps[:, :],
                                bq_tiles[(k, m)][:, :],
                                w_tiles[k][:, :],
                                start=(k == 0),
                                stop=(k == KT - 1),
                            )
                        qe = qepool.tile([P, NW], F32, name="qe")
                        nc.vector.tensor_add(
                            qe[:, :], ps[:, :], bias_t[:, j * NW:(j + 1) * NW]
                        )
                        nc.sync.dma_start(
                            out=ain[j][m * P:(m + 1) * P, :].bitcast(F32), in_=qe[:, :]
                        )

            def a2a(j):
                nc.gpsimd.collective_compute(
                    "AllToAll",
                    mybir.AluOpType.bypass,
                    replica_groups=[list(range(NC))],
                    ins=[ain[j].opt()],
                    outs=[aout[j].opt()],
                )

            def stage2(j):
                # (t, r, d) view ordered to match q2's partition-major
                # flat order p = tt*8 + r  (all permutation on DRAM side)
                ao = aout[j].rearrange("(r t) d -> t r d", r=NC)  # (128,8,512)
                with nc.named_scope(f"s2_q{j}"):
                    for g in range(TG):
                        q2 = q2pool.tile([P, NW], F32R, name="q2")
                        nc.sync.dma_start(
                            out=q2[:, :],
                            in_=ao[g * 16:(g + 1) * 16, :, :],
                        )
                        for h in range(BG):
                            ps2 = ps2pool.tile([P, NW], F32, name="ps2")
                            nc.tensor.matmul(
                                ps2[:, :],
                                l_tiles[(g, h)][:, :],
                                q2[:, :],
                                start=True,
                                stop=True,
                            )
                            o2 = o2pool.tile([P, NW], F32, name="o2")
                            if h % 2 == 0:
                                nc.scalar.copy(o2[:, :], ps2[:, :])
                            else:
                                nc.vector.tensor_copy(o2[:, :], ps2[:, :])
                            nc.sync.dma_start(
                                out=dq[h * 8:(h + 1) * 8,
                                       g * 16:(g + 1) * 16,
                                       j * NW:(j + 1) * NW]
                                .rearrange("b t n -> t b n"),
                                in_=o2[:, :],
                            )

            # Emission order staggers stage2(j) after stage1(j+1) so the
            # AllToAll latency hides under the next quarter's PE work.
            stage1(0)
            a2a(0)
            for j in range(1, NQ):
                stage1(j)
                stage2(j - 1)
                a2a(j)
            stage2(NQ - 1)

    nc.finalize()
    return nc


def _prep_inputs(query_weights, basic_queries, W_mlp, b_mlp):
    qw = np.ascontiguousarray(query_weights, dtype=np.float32)
    bq = np.ascontiguousarray(basic_queries, dtype=np.float32)
    W = np.ascontiguousarray(W_mlp, dtype=np.float32)
    b = np.ascontiguousarray(b_mlp, dtype=np.float32)

    bqT = np.ascontiguousarray(bq.T)  # (D, T), shared by all cores

    g_i = np.arange(TG)[:, None, None, None, None]
    h_i = np.arange(BG)[None, :, None, None, None]
    tt_i = np.arange(16)[None, None, :, None, None]
    r_i = np.arange(R)[None, None, None, :, None]
    bb_i = np.arange(8)[None, None, None, None, :]

    in_maps = []
    for c in range(NC):
        Wc = np.ascontiguousarray(W[:, c * D:(c + 1) * D])
        biasr = np.ascontiguousarray(
            np.broadcast_to(b[c * D:(c + 1) * D], (P, D))
        )
        qw_c = qw[:, c * TS:(c + 1) * TS, :]  # (32, 128, 8)
        L = np.zeros((TG, BG, P, P), np.float32)
        L[g_i, h_i, tt_i * 8 + r_i, tt_i * 8 + bb_i] = \
            qw_c[h_i * 8 + bb_i, g_i * 16 + tt_i, r_i]
        in_maps.append({"bqT": bqT, "Wc": Wc, "biasr": biasr, "Lt": L})
    return in_maps


last_results = None  # exposed for external profiling harnesses


def kernel(query_weights, basic_queries, W_mlp, b_mlp):
    global last_results
    if "nc" not in _cache:
        _cache["nc"] = _build_nc()
    nc = _cache["nc"]

    in_maps = _prep_inputs(query_weights, basic_queries, W_mlp, b_mlp)
    res = run_bass_kernel_spmd(nc, in_maps, core_ids=list(range(NC)))
    last_results = res

    dq = np.concatenate([res.results[c]["dq"] for c in range(NC)], axis=1)
    basic_expanded = np.broadcast_to(
        np.ascontiguousarray(basic_queries, dtype=np.float32)[None], (B, T, D)
    )
    return dq, basic_expanded


# revision 10
# speedup vs baseline: 1.0799x; 1.0799x over previous
"""Trainium2 Bass kernel for nn_DynamicDictionaryLearning (vq_codebook).

Computation (full shapes):
    query_embed = (basic_queries @ W_mlp + b_mlp).reshape(T, R, D)    # (T, R*D)
    dynamic_queries = einsum('btr,trd->btd', query_weights, query_embed)
    basic_expanded  = broadcast(basic_queries, (B, T, D))

Sharding (8 NeuronCores, one chip):
    Stage 1 (token-MLP expansion) is tensor-sharded over the R*D output dim:
    core r computes qe_r = basic_queries @ W_mlp[:, r*D:(r+1)*D] + b_r for
    ALL tokens, reading only 1/8th of W_mlp per core.

    An on-chip AllToAll redistributes qe so core c holds all R slices for
    its 128-token slice.  Stage 2 (weighted sum over R) runs as dense PE
    matmuls with block-diagonal qw tiles: contraction packs (16 tokens x
    8 r) = 128, output packs (16 tokens x 8 batch) = 128.

    The pipeline is chunked over 4 D-quarters so collectives and output
    DMA overlap the next quarter's stage-1 matmuls.  Quarter 0 runs the
    contraction loop outermost (8 PSUM banks) so the PE starts as soon as
    the first (bq, W) tile pair lands instead of waiting for the full
    preload.

    basic_expanded is a pure broadcast of an input -> host-side view.
"""

import os

import numpy as np
import ml_dtypes

import concourse.bass as bass
import concourse.mybir as mybir
import concourse.tile as tile
from concourse import bacc
from concourse.bass_utils import run_bass_kernel_spmd

# Problem shapes (hardcoded per spec)
D = 2048
T = 1024
R = 8
B = 32
NC = 8
TS = T // NC          # 128 tokens per core (stage-2 ownership)
P = 128
KT = D // P           # 16 contraction tiles
MT = T // P           # 8 token tiles (stage 1)
NQ = 4                # D-quarters (pipeline chunks)
NW = D // NQ          # 512 = matmul free dim / PSUM bank
TG = TS // 16         # 8 token groups of 16 (stage 2)
BG = B // 8           # 4 batch groups of 8 (stage 2)

F32 = mybir.dt.float32
F32R = mybir.dt.float32r
BF16 = mybir.dt.bfloat16

# matmul-operand dtype: "f32r" (full fp32 data, ~1e-4 rel err) or
# "bf16" (half the DMA traffic, ~4e-3 rel err)
USE_BF16 = os.environ.get("KBF16", "0") == "1"
DT_MM = BF16 if USE_BF16 else F32R
NP_MM = ml_dtypes.bfloat16 if USE_BF16 else np.float32

_cache = {}


def _build_nc():
    nc = bacc.Bacc("TRN2", target_bir_lowering=False, num_devices=NC)

    bqT = nc.dram_tensor("bqT", [D, T], DT_MM, kind="ExternalInput")
    Wc = nc.dram_tensor("Wc", [D, D], DT_MM, kind="ExternalInput")
    biasr = nc.dram_tensor("biasr", [P, D], F32, kind="ExternalInput")
    # block-diagonal qw tiles, packed (128, 32*128) for one big-line DMA
    Lt = nc.dram_tensor("Lt", [P, TG * BG * P], DT_MM, kind="ExternalInput")
    dq = nc.dram_tensor("dq", [B, TS, D], F32, kind="ExternalOutput")

    bqT_t = bqT.rearrange("(kt p) m -> kt p m", p=P)   # (16, 128, 1024)
    Wc_t = Wc.rearrange("(kt p) d -> kt p d", p=P)     # (16, 128, 2048)

    with tile.TileContext(nc) as tc:
        with (
            tc.tile_pool(name="bqp", bufs=1) as bqpool,
            tc.tile_pool(name="wp", bufs=2 * KT + 2) as wpool,
            tc.tile_pool(name="constp", bufs=1) as cpool,
            tc.tile_pool(name="qep", bufs=4) as qepool,
            tc.tile_pool(name="q2p", bufs=3) as q2pool,
            tc.tile_pool(name="o2p", bufs=6) as o2pool,
            tc.tile_pool(name="psp", bufs=8, space="PSUM") as pspool,
            tc.tile_pool(name="dramp", bufs=1, space="DRAM") as dram,
        ):
            # --- quarter-0 weights + bq, interleaved so the k-outer loop
            # below can start computing after the first pair lands ---
            bq_tiles = []
            w0_tiles = []
            for k in range(KT):
                wt = wpool.tile([P, NW], DT_MM, name="w")
                nc.sync.dma_start(out=wt, in_=Wc_t[k][:, 0:NW])
                w0_tiles.append(wt)
                bt = bqpool.tile([P, T], DT_MM, name=f"bq{k}")
                nc.sync.dma_start(out=bt, in_=bqT_t[k])
                bq_tiles.append(bt)
            bias_t = cpool.tile([P, D], F32, name="bias")
            nc.sync.dma_start(out=bias_t, in_=biasr[:, :])

            ain = [dram.tile([T, NW], DT_MM, name=f"ain{j}") for j in range(NQ)]
            aout = [dram.tile([T, NW], DT_MM, name=f"aout{j}") for j in range(NQ)]

            def qe_store(j, m, ps):
                qe = qepool.tile([P, NW], DT_MM, name="qe")
                nc.vector.tensor_add(
                    qe[:, :], ps[:, :], bias_t[:, j * NW:(j + 1) * NW]
                )
                nc.sync.dma_start(
                    out=ain[j][m * P:(m + 1) * P, :], in_=qe[:, :]
                )

            def stage1_stream(j):
                # k-outer: 8 concurrent PSUM accumulation chains, streams
                # (bq, W) tiles as they arrive.  Uses all 8 PSUM banks.
                ps = [pspool.tile([P, NW], F32, name="ps") for m in range(MT)]
                with nc.named_scope(f"s1_q{j}"):
                    for k in range(KT):
                        for m in range(MT):
                            nc.tensor.matmul(
                                ps[m][:, :],
                                bq_tiles[k][:, m * P:(m + 1) * P],
                                w0_tiles[k][:, :],
                                start=(k == 0),
                                stop=(k == KT - 1),
                            )
                    for m in range(MT):
                        qe_store(j, m, ps[m])

            def stage1(j):
                w_tiles = []
                for k in range(KT):
                    t = wpool.tile([P, NW], DT_MM, name="w")
                    nc.sync.dma_start(out=t, in_=Wc_t[k][:, j * NW:(j + 1) * NW])
                    w_tiles.append(t)
                with nc.named_scope(f"s1_q{j}"):
                    for m in range(MT):
                        ps = pspool.tile([P, NW], F32, name="ps")
                        for k in range(KT):
                            nc.tensor.matmul(
                                ps[:, :],
                                bq_tiles[k][:, m * P:(m + 1) * P],
                                w_tiles[k][:, :],
                                start=(k == 0),
                                stop=(k == KT - 1),
                            )
                        qe_store(j, m, ps)

            def a2a(j):
                nc.gpsimd.collective_compute(
                    "AllToAll",
                    mybir.AluOpType.bypass,
                    replica_groups=[list(range(NC))],
                    ins=[ain[j].opt()],
                    outs=[aout[j].opt()],
                )

            def stage2(j):
                # (t, r, d) view ordered to match q2's partition-major
                # flat order p = tt*8 + r  (all permutation on DRAM side)
                ao = aout[j].rearrange("(r t) d -> t r d", r=NC)  # (128,8,512)
                with nc.named_scope(f"s2_q{j}"):
                    for g in range(TG):
                        q2 = q2pool.tile([P, NW], DT_MM, name="q2")
                        nc.sync.dma_start(
                            out=q2[:, :],
                            in_=ao[g * 16:(g + 1) * 16, :, :],
                        )
                        for h in range(BG):
                            ps2 = pspool.tile([P, NW], F32, name="ps")
                            nc.tensor.matmul(
                                ps2[:, :],
                                l_tiles[(g, h)][:, :],
                                q2[:, :],
                                start=True,
                                stop=True,
                            )
                            o2 = o2pool.tile([P, NW], F32, name="o2")
                            if h % 2 == 0:
                                nc.scalar.copy(o2[:, :], ps2[:, :])
                            else:
                                nc.vector.tensor_copy(o2[:, :], ps2[:, :])
                            nc.sync.dma_start(
                                out=dq[h * 8:(h + 1) * 8,
                                       g * 16:(g + 1) * 16,
                                       j * NW:(j + 1) * NW]
                                .rearrange("b t n -> t b n"),
                                in_=o2[:, :],
                            )

            # quarter 0 streams; stage2(j) staggered after stage1(j+1) so
            # each AllToAll hides under the next quarter's PE work.
            stage1_stream(0)
            a2a(0)
            # L tiles (stage-2 weights) load after quarter 0 is in flight
            lbig = cpool.tile([P, TG * BG * P], DT_MM, name="lbig")
            nc.sync.dma_start(out=lbig, in_=Lt[:, :])
            l_tiles = {
                (g, h): lbig[:, (g * BG + h) * P:(g * BG + h + 1) * P]
                for g in range(TG)
                for h in range(BG)
            }
            for j in range(1, NQ):
                stage1(j)
                stage2(j - 1)
                a2a(j)
            stage2(NQ - 1)

    nc.finalize()
    return nc


def _prep_inputs(query_weights, basic_queries, W_mlp, b_mlp):
    qw = np.ascontiguousarray(query_weights, dtype=np.float32)
    bq = np.ascontiguousarray(basic_queries, dtype=np.float32)
    W = np.ascontiguousarray(W_mlp, dtype=np.float32)
    b = np.ascontiguousarray(b_mlp, dtype=np.float32)

    bqT = np.ascontiguousarray(bq.T.astype(NP_MM))  # (D, T), shared

    g_i = np.arange(TG)[:, None, None, None, None]
    h_i = np.arange(BG)[None, :, None, None, None]
    tt_i = np.arange(16)[None, None, :, None, None]
    r_i = np.arange(R)[None, None, None, :, None]
    bb_i = np.arange(8)[None, None, None, None, :]

    in_maps = []
    for c in range(NC):
        Wc = np.ascontiguousarray(W[:, c * D:(c + 1) * D].astype(NP_MM))
        biasr = np.ascontiguousarray(
            np.broadcast_to(b[c * D:(c + 1) * D], (P, D))
        )
        qw_c = qw[:, c * TS:(c + 1) * TS, :]  # (32, 128, 8)
        L = np.zeros((TG, BG, P, P), NP_MM)
        L[g_i, h_i, tt_i * 8 + r_i, tt_i * 8 + bb_i] = \
            qw_c[h_i * 8 + bb_i, g_i * 16 + tt_i, r_i].astype(NP_MM)
        # pack to (128, 32*128): Lbig[p, (g*BG+h)*128 + m] = L[g, h, p, m]
        Lbig = np.ascontiguousarray(
            L.transpose(2, 0, 1, 3).reshape(P, TG * BG * P)
        )
        in_maps.append({"bqT": bqT, "Wc": Wc, "biasr": biasr, "Lt": Lbig})
    return in_maps


last_results = None  # exposed for external profiling harnesses


def kernel(query_weights, basic_queries, W_mlp, b_mlp):
    global last_results
    if "nc" not in _cache:
        _cache["nc"] = _build_nc()
    nc = _cache["nc"]

    in_maps = _prep_inputs(query_weights, basic_queries, W_mlp, b_mlp)
    res = run_bass_kernel_spmd(nc, in_maps, core_ids=list(range(NC)))
    last_results = res

    dq = np.concatenate([res.results[c]["dq"] for c in range(NC)], axis=1)
    basic_expanded = np.broadcast_to(
        np.ascontiguousarray(basic_queries, dtype=np.float32)[None], (B, T, D)
    )
    return dq, basic_expanded


# revision 11
# speedup vs baseline: 1.2119x; 1.1223x over previous
"""Trainium2 Bass kernel for nn_DynamicDictionaryLearning (vq_codebook).

Computation (full shapes):
    query_embed = (basic_queries @ W_mlp + b_mlp).reshape(T, R, D)    # (T, R*D)
    dynamic_queries = einsum('btr,trd->btd', query_weights, query_embed)
    basic_expanded  = broadcast(basic_queries, (B, T, D))

Sharding (8 NeuronCores, one chip):
    Stage 1 (token-MLP expansion) is tensor-sharded over the R*D output dim:
    core r computes qe_r = basic_queries @ W_mlp[:, r*D:(r+1)*D] + b_r for
    ALL tokens, reading only 1/8th of W_mlp per core.

    An on-chip AllToAll redistributes qe so core c holds all R slices for
    its 128-token slice.  Stage 2 (weighted sum over R) runs as dense PE
    matmuls with block-diagonal qw tiles: contraction packs (16 tokens x
    8 r) = 128, output packs (16 tokens x 8 batch) = 128.

    The pipeline is chunked over 4 D-quarters so collectives and output
    DMA overlap the next quarter's stage-1 matmuls.  Quarter 0 runs the
    contraction loop outermost (8 PSUM banks) so the PE starts as soon as
    the first (bq, W) tile pair lands instead of waiting for the full
    preload.

    basic_expanded is a pure broadcast of an input -> host-side view.
"""

import os

import numpy as np
import ml_dtypes

import concourse.bass as bass
import concourse.mybir as mybir
import concourse.tile as tile
from concourse import bacc
from concourse.bass_utils import run_bass_kernel_spmd

# Problem shapes (hardcoded per spec)
D = 2048
T = 1024
R = 8
B = 32
NC = 8
TS = T // NC          # 128 tokens per core (stage-2 ownership)
P = 128
KT = D // P           # 16 contraction tiles
MT = T // P           # 8 token tiles (stage 1)
NQ = 4                # D-quarters (pipeline chunks)
NW = D // NQ          # 512 = matmul free dim / PSUM bank
TG = TS // 16         # 8 token groups of 16 (stage 2)
BG = B // 8           # 4 batch groups of 8 (stage 2)

F32 = mybir.dt.float32
F32R = mybir.dt.float32r
BF16 = mybir.dt.bfloat16

# matmul-operand dtype: "f32r" (full fp32 data, ~1e-4 rel err) or
# "bf16" (half the DMA traffic, ~4e-3 rel err)
USE_BF16 = os.environ.get("KBF16", "0") == "1"
DT_MM = BF16 if USE_BF16 else F32R
NP_MM = ml_dtypes.bfloat16 if USE_BF16 else np.float32

_cache = {}


def _build_nc():
    nc = bacc.Bacc("TRN2", target_bir_lowering=False, num_devices=NC)

    bqT = nc.dram_tensor("bqT", [D, T], DT_MM, kind="ExternalInput")
    Wc = nc.dram_tensor("Wc", [D, D], DT_MM, kind="ExternalInput")
    biasr = nc.dram_tensor("biasr", [P, D], F32, kind="ExternalInput")
    # block-diagonal qw tiles, packed (128, 32*128) for one big-line DMA
    Lt = nc.dram_tensor("Lt", [P, TG * BG * P], DT_MM, kind="ExternalInput")
    dq = nc.dram_tensor("dq", [B, TS, D], F32, kind="ExternalOutput")

    bqT_t = bqT.rearrange("(kt p) m -> kt p m", p=P)   # (16, 128, 1024)
    Wc_t = Wc.rearrange("(kt p) d -> kt p d", p=P)     # (16, 128, 2048)

    with tile.TileContext(nc) as tc:
        with (
            tc.tile_pool(name="bqp", bufs=1) as bqpool,
            tc.tile_pool(name="wp", bufs=2 * KT + 2) as wpool,
            tc.tile_pool(name="constp", bufs=1) as cpool,
            tc.tile_pool(name="qep", bufs=4) as qepool,
            tc.tile_pool(name="q2p", bufs=3) as q2pool,
            tc.tile_pool(name="o2p", bufs=6) as o2pool,
            tc.tile_pool(name="psp", bufs=8, space="PSUM") as pspool,
            tc.tile_pool(name="dramp", bufs=1, space="DRAM") as dram,
        ):
            # --- quarter-0 weights + bq, interleaved so the k-outer loop
            # below can start computing after the first pair lands ---
            bq_tiles = []
            w0_tiles = []
            for k in range(KT):
                wt = wpool.tile([P, NW], DT_MM, name="w")
                nc.sync.dma_start(out=wt, in_=Wc_t[k][:, 0:NW])
                w0_tiles.append(wt)
                bt = bqpool.tile([P, T], DT_MM, name=f"bq{k}")
                nc.sync.dma_start(out=bt, in_=bqT_t[k])
                bq_tiles.append(bt)
            bias_t = cpool.tile([P, D], F32, name="bias")
            nc.sync.dma_start(out=bias_t, in_=biasr[:, :])

            ain = [dram.tile([T, NW], DT_MM, name=f"ain{j}") for j in range(NQ)]
            aout = [dram.tile([T, NW], DT_MM, name=f"aout{j}") for j in range(NQ)]

            def qe_store(j, m, ps):
                qe = qepool.tile([P, NW], DT_MM, name="qe")
                nc.vector.tensor_add(
                    qe[:, :], ps[:, :], bias_t[:, j * NW:(j + 1) * NW]
                )
                nc.sync.dma_start(
                    out=ain[j][m * P:(m + 1) * P, :], in_=qe[:, :]
                )

            def stage1_stream(j):
                # k-outer: 8 concurrent PSUM accumulation chains, streams
                # (bq, W) tiles as they arrive.  Uses all 8 PSUM banks.
                ps = [pspool.tile([P, NW], F32, name="ps") for m in range(MT)]
                with nc.named_scope(f"s1_q{j}"):
                    for k in range(KT):
                        for m in range(MT):
                            nc.tensor.matmul(
                                ps[m][:, :],
                                bq_tiles[k][:, m * P:(m + 1) * P],
                                w0_tiles[k][:, :],
                                start=(k == 0),
                                stop=(k == KT - 1),
                            )
                    for m in range(MT):
                        qe_store(j, m, ps[m])

            def stage1(j):
                w_tiles = []
                for k in range(KT):
                    t = wpool.tile([P, NW], DT_MM, name="w")
                    nc.sync.dma_start(out=t, in_=Wc_t[k][:, j * NW:(j + 1) * NW])
                    w_tiles.append(t)
                with nc.named_scope(f"s1_q{j}"):
                    for m in range(MT):
                        ps = pspool.tile([P, NW], F32, name="ps")
                        for k in range(KT):
                            nc.tensor.matmul(
                                ps[:, :],
                                bq_tiles[k][:, m * P:(m + 1) * P],
                                w_tiles[k][:, :],
                                start=(k == 0),
                                stop=(k == KT - 1),
                            )
                        qe_store(j, m, ps)

            def a2a(j):
                nc.gpsimd.collective_compute(
                    "AllToAll",
                    mybir.AluOpType.bypass,
                    replica_groups=[list(range(NC))],
                    ins=[ain[j].opt()],
                    outs=[aout[j].opt()],
                )

            def stage2(j):
                # (t, r, d) view ordered to match q2's partition-major
                # flat order p = tt*8 + r  (all permutation on DRAM side)
                ao = aout[j].rearrange("(r t) d -> t r d", r=NC)  # (128,8,512)
                with nc.named_scope(f"s2_q{j}"):
                    for g in range(TG):
                        q2 = q2pool.tile([P, NW], DT_MM, name="q2")
                        nc.sync.dma_start(
                            out=q2[:, :],
                            in_=ao[g * 16:(g + 1) * 16, :, :],
                        )
                        for h in range(BG):
                            ps2 = pspool.tile([P, NW], F32, name="ps")
                            nc.tensor.matmul(
                                ps2[:, :],
                                l_tiles[(g, h)][:, :],
                                q2[:, :],
                                start=True,
                                stop=True,
                            )
                            o2 = o2pool.tile([P, NW], F32, name="o2")
                            if h % 2 == 0:
                                nc.scalar.copy(o2[:, :], ps2[:, :])
                            else:
                                nc.vector.tensor_copy(o2[:, :], ps2[:, :])
                            nc.sync.dma_start(
                                out=dq[h * 8:(h + 1) * 8,
                                       g * 16:(g + 1) * 16,
                                       j * NW:(j + 1) * NW]
                                .rearrange("b t n -> t b n"),
                                in_=o2[:, :],
                            )

            # quarter 0 streams; stage2(j) staggered after stage1(j+1) so
            # each AllToAll hides under the next quarter's PE work.
            stage1_stream(0)
            a2a(0)
            # L tiles (stage-2 weights) load after quarter 0 is in flight
            lbig = cpool.tile([P, TG * BG * P], DT_MM, name="lbig")
            nc.sync.dma_start(out=lbig, in_=Lt[:, :])
            l_tiles = {
                (g, h): lbig[:, (g * BG + h) * P:(g * BG + h + 1) * P]
                for g in range(TG)
                for h in range(BG)
            }
            # stage2(j) delayed two quarters behind stage1 so each
            # AllToAll has ~2 quarters of PE work to hide under.
            stage1(1)
            a2a(1)
            stage1(2)
            a2a(2)
            stage2(0)
            stage1(3)
            a2a(3)
            for j in range(1, NQ):
                stage2(j)

    nc.finalize()
    return nc


def _prep_inputs(query_weights, basic_queries, W_mlp, b_mlp):
    qw = np.ascontiguousarray(query_weights, dtype=np.float32)
    bq = np.ascontiguousarray(basic_queries, dtype=np.float32)
    W = np.ascontiguousarray(W_mlp, dtype=np.float32)
    b = np.ascontiguousarray(b_mlp, dtype=np.float32)

    bqT = np.ascontiguousarray(bq.T.astype(NP_MM))  # (D, T), shared

    g_i = np.arange(TG)[:, None, None, None, None]
    h_i = np.arange(BG)[None, :, None, None, None]
    tt_i = np.arange(16)[None, None, :, None, None]
    r_i = np.arange(R)[None, None, None, :, None]
    bb_i = np.arange(8)[None, None, None, None, :]

    in_maps = []
    for c in range(NC):
        Wc = np.ascontiguousarray(W[:, c * D:(c + 1) * D].astype(NP_MM))
        biasr = np.ascontiguousarray(
            np.broadcast_to(b[c * D:(c + 1) * D], (P, D))
        )
        qw_c = qw[:, c * TS:(c + 1) * TS, :]  # (32, 128, 8)
        L = np.zeros((TG, BG, P, P), NP_MM)
        L[g_i, h_i, tt_i * 8 + r_i, tt_i * 8 + bb_i] = \
            qw_c[h_i * 8 + bb_i, g_i * 16 + tt_i, r_i].astype(NP_MM)
        # pack to (128, 32*128): Lbig[p, (g*BG+h)*128 + m] = L[g, h, p, m]
        Lbig = np.ascontiguousarray(
            L.transpose(2, 0, 1, 3).reshape(P, TG * BG * P)
        )
        in_maps.append({"bqT": bqT, "Wc": Wc, "biasr": biasr, "Lt": Lbig})
    return in_maps


last_results = None  # exposed for external profiling harnesses


def kernel(query_weights, basic_queries, W_mlp, b_mlp):
    global last_results
    if "nc" not in _cache:
        _cache["nc"] = _build_nc()
    nc = _cache["nc"]

    in_maps = _prep_inputs(query_weights, basic_queries, W_mlp, b_mlp)
    res = run_bass_kernel_spmd(nc, in_maps, core_ids=list(range(NC)))
    last_results = res

    dq = np.concatenate([res.results[c]["dq"] for c in range(NC)], axis=1)
    basic_expanded = np.broadcast_to(
        np.ascontiguousarray(basic_queries, dtype=np.float32)[None], (B, T, D)
    )
    return dq, basic_expanded
